# revision 30
# baseline (speedup 1.0000x reference)
"""Trainium2 Bass kernel for DescartesExtension (order-2 polynomial feature map).

reference: out[b, n(i,j)] = x[b,i] * x[b,j] for i<=j in row-major upper-tri order,
x: [256, 1024] f32 -> out: [256, 524800] f32.

Structure used: for fixed i, output columns [off(i), off(i)+D-i) are
x[b,i] * x[b, i:D] -- a per-partition scalar times a contiguous slice
(tensor_scalar_mul on the DVE / activation-with-scale on ACT, batch rows on
partitions).

Sharding (SPMD: one program, 8 cores, per-core differences only in input data):
core c handles segments i = c + 8k, k = 0..127.  Slot k runs a UNIFORM-width op
T_k = 1024 - 8k on a host-shifted input row xs_c[b, t] = x[b, t+c] (zero
padded), so every AP in the program is identical across cores.  Core c's slot k
therefore computes its segment (length T_k - c) plus c trailing zeros.  Each
core writes a packed private output [256, 66048]; the host scatters slots back
into the full output and drops the padding tails.

Precision: the tensor-operand pipeline runs in bf16 (bf16 stores -> half the
HBM/fabric store traffic of f32); the per-partition scalars stay exact f32.
The host upcasts the gathered result to f32.  Measured max relative error
7.8e-3, well under the 2e-2 gate.

Input packing: each 128-row block's inputs arrive as ONE uint8 tensor row of
2560 bytes -- 512B f32 scalars (xfs[b,k] = x[b, c+8k]) followed by 2048B bf16
shifted row -- read on-chip through bitcast APs.  Block 0's load is split so
the first piece (scalars + first 128 bf16 cols, 768B/row) lands ~0.7us before
the rest; the first output chunk needs only that piece, so the first store
triggers right after it arrives.

Chunking: compute granularity (per-slot tensor_scalar ops, slots split at
chunk boundaries) is decoupled from store granularity.  Stores use a
geometric ramp of windows (128 .. 13K cols, pipeline fill at ~1.35x the
drain rate) and then steady 16384-col windows = 32KB per-partition DMA
packets, 4-deep buffered.  Measured steady state: each of the 16 SDMA
engines runs at ~26.9 GB/s (~99% of its 27.2 GB/s AXI-port share, any
packet size >= 27KB), i.e. ~430 GB/s of stores per core with zero
inter-packet gaps after the ramp.  The final window of each block is small
(tapered) so the post-compute drain is short.  Compute is split DVE
(tensor_scalar packed 16-bit mode, ~76%) + ACT (activation Copy with
per-partition f32 scale, wide slots only, ~24%) so combined production
(~1.3x the drain rate) keeps the store queue saturated.

Known environmental hazard (not controllable from the kernel): on this
part, ports 15 of NCs {0,4} and ports 0 of NCs {2,6} are intermittently
capped at ~21 GB/s for a whole run (packet-size independent), which adds
~17us to any afflicted core.  Chunk geometry, queue striping, and packet
size were all measured to have no effect on it; the kernel instead
minimizes the serial lead-in/tail around the store stream, which bounds
both the clean case (~97us) and the afflicted case (~113us).
"""

import numpy as np
import ml_dtypes

B = 256
D = 1024
NCORES = 8
NSLOT = D // NCORES  # 128 slots per core
T = [D - NCORES * k for k in range(NSLOT)]  # uniform slot widths 1024, 1016, ..., 8
S = [0] * (NSLOT + 1)  # packed slot offsets
for _k in range(NSLOT):
    S[_k + 1] = S[_k] + T[_k]
OUTW = S[NSLOT]  # 66048 packed columns per core
SCB = 4 * NSLOT  # 512 bytes of f32 scalars at the front of each packed row
ROWB = SCB + 2 * D  # 2560 packed input bytes per row
SPLIT0 = 128  # bf16 cols in block-0's first load piece (with the scalars)
CHUNK_MAX = 16384  # tile width = steady store window (32KB/partition bf16)
BUFS = 4  # chunk buffering depth (4 x 32KB = 128KB/partition)
STRIPE = 0  # 1: alternate steady-chunk stores across both HWDGE rings
# pipeline-fill store windows for block 0 (cols); geometric, ~1.35x growth
# headroom vs the drain rate, ending slot-aligned at S[32]=28800
RAMP = (128, 1024, 2040, 4064, 8016, 15536, 28800)
ACT_MOD = 4  # 1-in-ACT_MOD wide slots run on the ACT engine

# --- risk-balanced store layout: see memory notes.  Engines 0 and 15 are
# intermittently capped at ~21 GB/s; for the CUT window the main stores skip
# their partitions (3 subrange DMAs spread across the sync/scalar/gpsimd DGE
# paths so they drain concurrently), and the 16 excluded rows per block get
# those columns from a displaced 96-partition pipeline (xin2 -> out2).
GW = 5108
CUT0 = 32768
CUT1 = CUT0 + 3 * GW  # 48092
SAFE = ((4, 32), (36, 92), (96, 124))
AF = [p for lo, hi in ((0, 4), (32, 36), (92, 96), (124, 128))
      for p in range(lo, hi)]
AF = AF + [128 + p for p in AF]

_prog_cache = None


def _slot_of(col):
    import bisect
    return bisect.bisect_right(S, col) - 1


def _disp_ops():
    cuts = {0, GW}
    for g in range(3):
        cg = CUT0 + g * GW
        for k, a_, b_ in _pieces(cg, cg + GW):
            cuts.add(S[k] + a_ - cg)
    cs = sorted(cuts)
    return [(a_, b_, tuple(_slot_of(CUT0 + g * GW + a_) for g in range(3)))
            for a_, b_ in zip(cs, cs[1:])]


def _act_slot(k):
    """Slots handed to the ACT engine (~24% of packed columns).

    Only wide slots (4 < k < 96): ACT's ~224-cycle fixed cost per op makes it
    a poor fit for the narrow tail slots, and the first ramp chunks stay on
    the DVE so the pipeline fill isn't gated on the ACT table load."""
    return k % ACT_MOD == 1 and 4 < k < 96


def _windows(blk):
    """Store-window column boundaries for a 128-row block."""
    cuts = (list(RAMP) if blk == 0 else [16384]) + [CUT0, CUT1, 64000, OUTW]
    return list(zip([0] + cuts[:-1], cuts))


def _pieces(lo, hi):
    """Slot pieces (slot k, col a, col b within slot) covering packed output
    columns [lo, hi)."""
    out = []
    for k in range(NSLOT):
        a, b = max(lo, S[k]), min(hi, S[k + 1])
        if a < b:
            out.append((k, a - S[k], b - S[k]))
    return out


DISP = _disp_ops()
NDOP = len(DISP)
ROWB2 = 4 * NDOP + 2 * GW


def _build_program():
    global _prog_cache
    if _prog_cache is not None:
        return _prog_cache

    import concourse.bacc as bacc
    import concourse.mybir as mybir
    import concourse.tile as tile

    nc = bacc.Bacc("TRN2", target_bir_lowering=False, debug=False,
                   enable_partition_id=False)
    xin = nc.dram_tensor("xin", [B, ROWB], mybir.dt.uint8,
                         kind="ExternalInput").ap()
    xin2 = nc.dram_tensor("xin2", [128, ROWB2], mybir.dt.uint8,
                          kind="ExternalInput").ap()
    out = nc.dram_tensor("out", [B, OUTW], mybir.dt.bfloat16,
                         kind="ExternalOutput").ap()
    out2 = nc.dram_tensor("out2", [96, GW], mybir.dt.bfloat16,
                          kind="ExternalOutput").ap()

    with tile.TileContext(nc) as tc:
        with (
            tc.tile_pool(name="xp", bufs=1) as xp,
            tc.tile_pool(name="op", bufs=BUFS) as op,
        ):
            xt = [xp.tile([128, ROWB], mybir.dt.uint8, tag=f"x{b}", name=f"x{b}")
                  for b in range(2)]
            # Block 0 arrives in two pieces: the scalars + first SPLIT0 bf16
            # cols ride the sync ring (ahead of the stores), the rest + all of
            # block 1 ride the scalar ring.  The first compute only needs
            # piece A, so the first store triggers ~1.5us after the preamble.
            acut = SCB + 2 * SPLIT0
            nc.sync.dma_start(xt[0][:, :acut], xin[0:128, :acut])
            nc.scalar.dma_start(xt[0][:, acut:], xin[0:128, acut:])
            nc.scalar.dma_start(xt[1][:], xin[128:256, :])
            x2t = xp.tile([128, ROWB2], mybir.dt.uint8, tag="x2", name="x2")
            nc.scalar.dma_start(x2t[:], xin2[:, :])
            sc = [t[:, :SCB].bitcast(mybir.dt.float32) for t in xt]
            xb = [t[:, SCB:].bitcast(mybir.dt.bfloat16) for t in xt]
            sc2 = x2t[:, : 4 * NDOP].bitcast(mybir.dt.float32)
            xb2 = x2t[:, 4 * NDOP :].bitcast(mybir.dt.bfloat16)
            dt = xp.tile([128, GW], mybir.dt.bfloat16, tag="disp", name="disp")
            for blk in range(B // 128):
                for wi, (lo, hi) in enumerate(_windows(blk)):
                    w = hi - lo
                    pt = op.tile([128, CHUNK_MAX], mybir.dt.bfloat16, tag="packed")
                    for k, a, b_ in _pieces(lo, hi):
                        o = S[k] + a - lo
                        src = xb[blk][:, NCORES * k + a : NCORES * k + b_]
                        if _act_slot(k):
                            nc.scalar.mul(pt[:, o : o + b_ - a], src,
                                          sc[blk][:, k : k + 1])
                        else:
                            nc.vector.tensor_scalar_mul(
                                out=pt[:, o : o + b_ - a],
                                in0=src,
                                scalar1=sc[blk][:, k : k + 1],
                            )
                    if (lo, hi) == (CUT0, CUT1):
                        # risky engines sit out; three subrange stores on
                        # three DGE paths so they drain concurrently
                        for (plo, phi), e in zip(SAFE,
                                                 (nc.scalar, nc.sync,
                                                  nc.gpsimd)):
                            e.dma_start(
                                out[blk * 128 + plo : blk * 128 + phi, lo:hi],
                                pt[plo:phi, :w],
                            )
                    else:
                        nc.sync.dma_start(
                            out[blk * 128 : (blk + 1) * 128, lo:hi], pt[:, :w]
                        )
                    if blk == 1 and wi == 1:
                        for oi, (a_, b_, _ks) in enumerate(DISP):
                            nc.vector.tensor_scalar_mul(
                                out=dt[0:96, a_:b_],
                                in0=xb2[0:96, a_:b_],
                                scalar1=sc2[0:96, oi : oi + 1],
                            )
                        nc.sync.dma_start(out2[0:96, :], dt[0:96, :])
    nc.compile()
    _prog_cache = nc
    return nc


def _run(x, trace=False, trace_cores=None, tmpdir=None):
    """Returns (full_output, BassKernelResults)."""
    from concourse.bass_utils import run_bass_kernel_spmd

    x = np.ascontiguousarray(np.asarray(x), dtype=np.float32)
    assert x.shape == (B, D)
    nc = _build_program()

    xbf = x.astype(ml_dtypes.bfloat16)
    S_arr = np.array(S)
    cols = CUT0 + GW * np.arange(3)[:, None] + np.arange(GW)[None, :]
    kk = np.searchsorted(S_arr, cols, side="right") - 1
    tt = NCORES * kk + (cols - S_arr[kk])
    in_maps = []
    for c in range(NCORES):
        xsc = np.zeros((B, D), ml_dtypes.bfloat16)
        xsc[:, : D - c] = xbf[:, c:]
        # exact-f32 per-slot scalars: xfs[b, k] = x[b, c + 8k]
        xfc = np.ascontiguousarray(x[:, c::NCORES], np.float32)
        pk = np.empty((B, ROWB), np.uint8)
        pk[:, :SCB] = xfc.view(np.uint8)
        pk[:, SCB:] = xsc.view(np.uint8)
        xs2 = np.zeros((128, GW), ml_dtypes.bfloat16)
        sc2 = np.zeros((128, NDOP), np.float32)
        for g in range(3):
            xs2[g * 32 : g * 32 + 32] = xsc[AF][:, tt[g]]
            for oi, (_a, _b, ks) in enumerate(DISP):
                sc2[g * 32 : g * 32 + 32, oi] = xfc[AF, ks[g]]
        pk2 = np.empty((128, ROWB2), np.uint8)
        pk2[:, : 4 * NDOP] = sc2.view(np.uint8)
        pk2[:, 4 * NDOP :] = xs2.view(np.uint8)
        in_maps.append({"xin": pk, "xin2": pk2})

    kw = {}
    if tmpdir is not None:
        kw["tmpdir"] = tmpdir
    if trace:
        kw["trace"] = True
        if trace_cores is not None:
            kw["trace_cores"] = trace_cores
    res = run_bass_kernel_spmd(nc, in_maps, core_ids=list(range(NCORES)), **kw)

    off = np.zeros(D + 1, np.int64)
    off[1:] = np.cumsum(D - np.arange(D))
    full = np.empty((B, D * (D + 1) // 2), np.float32)
    for c in range(NCORES):
        r = np.array(res.results[c]["out"])
        r2 = res.results[c]["out2"]
        for g in range(3):
            r[AF, CUT0 + g * GW : CUT0 + (g + 1) * GW] = r2[g * 32 : g * 32 + 32]
        for k in range(NSLOT):
            i = c + NCORES * k
            L = D - i
            full[:, off[i] : off[i] + L] = r[:, S[k] : S[k] + L]
    return full, res


def kernel(x):
    return _run(x)[0]


# revision 31
# speedup vs baseline: 1.1221x; 1.1221x over previous
"""Trainium2 Bass kernel for DescartesExtension (order-2 polynomial feature map).

reference: out[b, n(i,j)] = x[b,i] * x[b,j] for i<=j in row-major upper-tri order,
x: [256, 1024] f32 -> out: [256, 524800] f32.

Structure used: for fixed i, output columns [off(i), off(i)+D-i) are
x[b,i] * x[b, i:D] -- a per-partition scalar times a contiguous slice
(tensor_scalar_mul on the DVE / activation-with-scale on ACT, batch rows on
partitions).

Sharding (SPMD: one program, 8 cores, per-core differences only in input data):
core c handles segments i = c + 8k, k = 0..127.  Slot k runs a UNIFORM-width op
T_k = 1024 - 8k on a host-shifted input row xs_c[b, t] = x[b, t+c] (zero
padded), so every AP in the program is identical across cores.  Core c's slot k
therefore computes its segment (length T_k - c) plus c trailing zeros.  Each
core writes a packed private output [256, 66048]; the host scatters slots back
into the full output and drops the padding tails.

Precision: the tensor-operand pipeline runs in bf16 (bf16 stores -> half the
HBM/fabric store traffic of f32); the per-partition scalars stay exact f32.
The host upcasts the gathered result to f32.  Measured max relative error
7.8e-3, well under the 2e-2 gate.

Input packing: each 128-row block's inputs arrive as ONE uint8 tensor row of
2560 bytes -- 512B f32 scalars (xfs[b,k] = x[b, c+8k]) followed by 2048B bf16
shifted row -- read on-chip through bitcast APs.  Block 0's load is split so
the first piece (scalars + first 128 bf16 cols, 768B/row) lands ~0.7us before
the rest; the first output chunk needs only that piece, so the first store
triggers right after it arrives.

Chunking: compute granularity (per-slot tensor_scalar ops, slots split at
chunk boundaries) is decoupled from store granularity.  Stores use a
geometric ramp of windows (128 .. 13K cols, pipeline fill at ~1.35x the
drain rate) and then steady 16384-col windows = 32KB per-partition DMA
packets, 4-deep buffered.  Measured steady state: each of the 16 SDMA
engines runs at ~26.9 GB/s (~99% of its 27.2 GB/s AXI-port share, any
packet size >= 27KB), i.e. ~430 GB/s of stores per core with zero
inter-packet gaps after the ramp.  The final window of each block is small
(tapered) so the post-compute drain is short.  Compute is split DVE
(tensor_scalar packed 16-bit mode, ~76%) + ACT (activation Copy with
per-partition f32 scale, wide slots only, ~24%) so combined production
(~1.3x the drain rate) keeps the store queue saturated.

Known environmental hazard (not controllable from the kernel): on this
part, ports 15 of NCs {0,4} and ports 0 of NCs {2,6} are intermittently
capped at ~21 GB/s for a whole run (packet-size independent), which adds
~17us to any afflicted core.  Chunk geometry, queue striping, and packet
size were all measured to have no effect on it; the kernel instead
minimizes the serial lead-in/tail around the store stream, which bounds
both the clean case (~97us) and the afflicted case (~113us).
"""

import numpy as np
import ml_dtypes

B = 256
D = 1024
NCORES = 8
NSLOT = D // NCORES  # 128 slots per core
T = [D - NCORES * k for k in range(NSLOT)]  # uniform slot widths 1024, 1016, ..., 8
S = [0] * (NSLOT + 1)  # packed slot offsets
for _k in range(NSLOT):
    S[_k + 1] = S[_k] + T[_k]
OUTW = S[NSLOT]  # 66048 packed columns per core
SCB = 4 * NSLOT  # 512 bytes of f32 scalars at the front of each packed row
ROWB = SCB + 2 * D  # 2560 packed input bytes per row
SPLIT0 = 128  # bf16 cols in block-0's first load piece (with the scalars)
CHUNK_MAX = 16384  # tile width = steady store window (32KB/partition bf16)
BUFS = 4  # chunk buffering depth (4 x 32KB = 128KB/partition)
STRIPE = 0  # 1: alternate steady-chunk stores across both HWDGE rings
# pipeline-fill store windows for block 0 (cols); geometric, ~1.35x growth
# headroom vs the drain rate, ending slot-aligned at S[32]=28800
RAMP = (128, 1024, 2040, 4064, 8016, 15536, 28800)
ACT_MOD = 4  # 1-in-ACT_MOD wide slots run on the ACT engine

_prog_cache = None


def _act_slot(k):
    """Slots handed to the ACT engine (~24% of packed columns).

    Only wide slots (4 < k < 96): ACT's ~224-cycle fixed cost per op makes it
    a poor fit for the narrow tail slots, and the first ramp chunks stay on
    the DVE so the pipeline fill isn't gated on the ACT table load."""
    return k % ACT_MOD == 1 and 4 < k < 96


def _windows(blk):
    """Store-window column boundaries for a 128-row block."""
    cuts = list(RAMP) if blk == 0 else []
    c = cuts[-1] if cuts else 0
    while c < OUTW:
        c = min(c + CHUNK_MAX, OUTW)
        cuts.append(c)
    # taper: if the final window is large, split off a small tail so the
    # post-compute drain is short
    if len(cuts) >= 2 and cuts[-1] - cuts[-2] > 8192:
        cuts.insert(-1, cuts[-1] - 2048)
    return list(zip([0] + cuts[:-1], cuts))


def _pieces(lo, hi):
    """Slot pieces (slot k, col a, col b within slot) covering packed output
    columns [lo, hi)."""
    out = []
    for k in range(NSLOT):
        a, b = max(lo, S[k]), min(hi, S[k + 1])
        if a < b:
            out.append((k, a - S[k], b - S[k]))
    return out


def _build_program():
    global _prog_cache
    if _prog_cache is not None:
        return _prog_cache

    import concourse.bacc as bacc
    import concourse.mybir as mybir
    import concourse.tile as tile

    nc = bacc.Bacc("TRN2", target_bir_lowering=False, debug=False,
                   enable_partition_id=False)
    xin = nc.dram_tensor("xin", [B, ROWB], mybir.dt.uint8,
                         kind="ExternalInput").ap()
    out = nc.dram_tensor("out", [B, OUTW], mybir.dt.bfloat16,
                         kind="ExternalOutput").ap()

    with tile.TileContext(nc) as tc:
        with (
            tc.tile_pool(name="xp", bufs=1) as xp,
            tc.tile_pool(name="op", bufs=BUFS) as op,
        ):
            xt = [xp.tile([128, ROWB], mybir.dt.uint8, tag=f"x{b}", name=f"x{b}")
                  for b in range(2)]
            # Block 0 arrives in two pieces: the scalars + first SPLIT0 bf16
            # cols ride the sync ring (ahead of the stores), the rest + all of
            # block 1 ride the scalar ring.  The first compute only needs
            # piece A, so the first store triggers ~1.5us after the preamble.
            acut = SCB + 2 * SPLIT0
            nc.sync.dma_start(xt[0][:, :acut], xin[0:128, :acut])
            nc.scalar.dma_start(xt[0][:, acut:], xin[0:128, acut:])
            nc.scalar.dma_start(xt[1][:], xin[128:256, :])
            sc = [t[:, :SCB].bitcast(mybir.dt.float32) for t in xt]
            xb = [t[:, SCB:].bitcast(mybir.dt.bfloat16) for t in xt]
            nstripe = 0
            for blk in range(B // 128):
                for lo, hi in _windows(blk):
                    w = hi - lo
                    pt = op.tile([128, CHUNK_MAX], mybir.dt.bfloat16, tag="packed")
                    for k, a, b_ in _pieces(lo, hi):
                        o = S[k] + a - lo
                        src = xb[blk][:, NCORES * k + a : NCORES * k + b_]
                        if _act_slot(k):
                            nc.scalar.mul(pt[:, o : o + b_ - a], src,
                                          sc[blk][:, k : k + 1])
                        else:
                            nc.vector.tensor_scalar_mul(
                                out=pt[:, o : o + b_ - a],
                                in0=src,
                                scalar1=sc[blk][:, k : k + 1],
                            )
                    eng = nc.sync
                    if STRIPE and w >= 8192:
                        nstripe += 1
                        if nstripe % 2 == 0:
                            eng = nc.scalar
                    eng.dma_start(
                        out[blk * 128 : (blk + 1) * 128, lo:hi], pt[:, :w]
                    )
    nc.compile()
    _prog_cache = nc
    return nc


def _run(x, trace=False, trace_cores=None, tmpdir=None):
    """Returns (full_output, BassKernelResults)."""
    from concourse.bass_utils import run_bass_kernel_spmd

    x = np.ascontiguousarray(np.asarray(x), dtype=np.float32)
    assert x.shape == (B, D)
    nc = _build_program()

    xbf = x.astype(ml_dtypes.bfloat16)
    in_maps = []
    for c in range(NCORES):
        xsc = np.zeros((B, D), ml_dtypes.bfloat16)
        xsc[:, : D - c] = xbf[:, c:]
        # exact-f32 per-slot scalars: xfs[b, k] = x[b, c + 8k]
        xfc = np.ascontiguousarray(x[:, c::NCORES], np.float32)
        pk = np.empty((B, ROWB), np.uint8)
        pk[:, :SCB] = xfc.view(np.uint8)
        pk[:, SCB:] = xsc.view(np.uint8)
        in_maps.append({"xin": pk})

    kw = {}
    if tmpdir is not None:
        kw["tmpdir"] = tmpdir
    if trace:
        kw["trace"] = True
        if trace_cores is not None:
            kw["trace_cores"] = trace_cores
    res = run_bass_kernel_spmd(nc, in_maps, core_ids=list(range(NCORES)), **kw)

    off = np.zeros(D + 1, np.int64)
    off[1:] = np.cumsum(D - np.arange(D))
    full = np.empty((B, D * (D + 1) // 2), np.float32)
    for c in range(NCORES):
        r = res.results[c]["out"]
        for k in range(NSLOT):
            i = c + NCORES * k
            L = D - i
            full[:, off[i] : off[i] + L] = r[:, S[k] : S[k] + L]
    return full, res


def kernel(x):
    return _run(x)[0]
